# revision 29
# baseline (speedup 1.0000x reference)
"""Trainium2 Bass kernel for nn_Attention_b (tanh-attention with masked_scatter).

Data-parallel over batch: each of 8 NeuronCores owns 4 batches. Per core:
  phase 1  z = W1 @ h_i + (W2 @ h_t + b)   (fp16 GEMM, [A, rows])
           m = tanh(z); y = u . m          (raw scores, [rows])
  comm     AllGather of the per-chunk score slice across the 8 cores
  phase 2  masked_scatter selection (0/1 matrix against gathered scores),
           e = exp(beta - C_b) with a per-batch shift C_b fixed from the
           first chunk's max (no flash recurrence needed)
  phase 3  weighted sum s += e * h_i, split across engines:
             h' in [0, 512):    PE matvec on DMA-transposed h_i tiles,
                                accumulated in persistent PSUM banks
             h' in [512, 1024): fused DVE multiply-reduce on the resident
                                GEMM-layout h_i chunk
"""
import sys

for _p in ("/opt/trn_rl_repo",):
    if _p not in sys.path:
        sys.path.insert(0, _p)

import numpy as np

import concourse.bacc as bacc
import concourse.tile as tile
from concourse import mybir
from concourse.bass_utils import run_bass_kernel_spmd
from concourse.dve_ops import TENSOR_TENSOR_REDUCE
from concourse.masks import make_identity

NCORES = 8
B, S, H, A = 32, 2048, 1024, 256
BL = B // NCORES          # local batches per core
NEG = np.float32(-60000.0)   # fp16-representable "minus infinity"
MARG = np.float32(4.0)       # shift margin over the chunk-0 max

f32 = mybir.dt.float32
f16 = mybir.dt.float16


def build_kernel(S=S, H=H, A=A, C=256, J=4, hi_bufs=7, hit_bufs=6,
                 clist=None, debug_taps=False):
    KT = H // 128             # contraction tiles
    AT = A // 128             # score tiles
    HJ = J * 128              # h-dims handled by the PE matvec half
    KTD = KT - J              # h-tiles handled by the DVE half
    ST = S // 128             # s-tiles for the transposed layout
    if clist is None:
        clist = [C] * (S // C)
    offs = np.concatenate([[0], np.cumsum(clist)]).tolist()
    NCH = len(clist)
    assert offs[-1] == S and H % 128 == 0 and A % 128 == 0
    assert all(c % 128 == 0 for c in clist)

    nc = bacc.Bacc("TRN2", target_bir_lowering=False, debug=False,
                   num_devices=NCORES)

    hi5 = nc.declare_dram_parameter("hi5", [128, KT * BL * S], f16,
                                    isOutput=False)
    hit5 = nc.declare_dram_parameter("hit5", [128, ST * BL * HJ], f16,
                                     isOutput=False)
    w1t = nc.declare_dram_parameter("w1t", [H, A], f16, isOutput=False)
    cb2 = nc.declare_dram_parameter("cb2", [128, AT, BL], f32, isOutput=False)
    u2 = nc.declare_dram_parameter("u2", [128, AT], f16, isOutput=False)
    sel = nc.declare_dram_parameter("sel", [B + 1, BL, S], f16,
                                    isOutput=False)
    out = nc.declare_dram_parameter("out", [BL, H], f32, isOutput=True)
    if debug_taps:
        ydbg = nc.declare_dram_parameter("ydbg", [B, S], f16, isOutput=True)
        btdbg = nc.declare_dram_parameter("btdbg", [1, BL, S], f32,
                                          isOutput=True)

    with tile.TileContext(nc) as tc:
        with (
            tc.tile_pool(name="consts", bufs=1) as cp,
            tc.tile_pool(name="hi", bufs=hi_bufs) as hip,
            tc.tile_pool(name="hit", bufs=hit_bufs) as hitp,
            tc.tile_pool(name="m", bufs=2) as mp,
            tc.tile_pool(name="small", bufs=3) as sp,
            tc.tile_pool(name="ebc", bufs=2) as ebp,
            tc.tile_pool(name="pz", bufs=2, space="PSUM") as pz,
            tc.tile_pool(name="py", bufs=2, space="PSUM") as py,
            tc.tile_pool(name="pm", bufs=1, space="PSUM") as pm,
            tc.tile_pool(name="dram", bufs=NCH, space="DRAM") as dp,
        ):
            # ---- preload replicated constants
            w1_sb = cp.tile([128, KT, A], f16)
            nc.sync.dma_start(
                out=w1_sb, in_=w1t.rearrange("(t p) a -> p t a", p=128))
            u_sb = cp.tile([128, AT], f16)
            nc.sync.dma_start(out=u_sb, in_=u2[:, :])
            cb_sb = cp.tile([128, AT, BL], f32)
            nc.sync.dma_start(out=cb_sb, in_=cb2[:, :, :])
            ident = cp.tile([128, 128], f32)
            make_identity(nc, ident)
            ones_sb = cp.tile([B + 1, 1], f16)
            nc.vector.memset(ones_sb, 1.0)
            one1 = cp.tile([1, 1], f16)
            nc.vector.memset(one1, 1.0)

            # per-chunk softmax l-partials; shared per-batch shift
            lall = cp.tile([1, BL, NCH], f32)
            nc.vector.memset(lall, 0.0)
            nCb = cp.tile([1, BL], f32)
            nc.vector.memset(nCb, 0.0)
            eT_sb = cp.tile([128, ST, BL], f16)
            saccs = [cp.tile([128, KTD, BL], f32, name=f"sacc{i}")
                     for i in range(NCH)]
            mv = [pm.tile([1, HJ], f32, name=f"mv{b}", tag=f"mv{b}")
                  for b in range(BL)]

            carries = []

            def phase1(i):
                Ci, off = clist[i], offs[i]
                TL = Ci // 128
                hi_sb = hip.tile([128, KT, BL, Ci], f16, tag="hi")
                nc.sync.dma_start(
                    out=hi_sb.rearrange("p t b s -> p (t b s)"),
                    in_=hi5[:, KT * BL * off : KT * BL * (off + Ci)])
                hit_sb = hitp.tile([128, TL, BL, HJ], f16, tag="hit")
                st0 = off // 128
                nc.sync.dma_start(
                    out=hit_sb.rearrange("p t b h -> p (t b h)"),
                    in_=hit5[:, st0 * BL * HJ : (st0 + TL) * BL * HJ])
                m_r = mp.tile([128, AT, BL, Ci], f16, tag="m")
                for at in range(AT):
                    for pr in range(BL // 2):
                        z_ps = pz.tile([128, 2, Ci], f32, tag="z")
                        for kt in range(KT):
                            nc.tensor.matmul(
                                z_ps,
                                w1_sb[:, kt, at * 128 : (at + 1) * 128],
                                hi_sb[:, kt, 2 * pr : 2 * pr + 2, :],
                                start=(kt == 0), stop=(kt == KT - 1),
                            )
                        for bh in range(2):
                            b = 2 * pr + bh
                            nc.scalar.activation(
                                out=m_r[:, at, b, :], in_=z_ps[:, bh, :],
                                func=mybir.ActivationFunctionType.Tanh,
                                bias=cb_sb[:, at, b : b + 1], scale=1.0,
                            )
                y_sb = sp.tile([1, BL, Ci], f16, tag="ysb", bufs=2)
                for pr in range(BL // 2):
                    y_ps = py.tile([1, 2, Ci], f32, tag="y")
                    for at in range(AT):
                        nc.tensor.matmul(
                            y_ps,
                            u_sb[:, at : at + 1],
                            m_r[:, at, 2 * pr : 2 * pr + 2, :],
                            start=(at == 0), stop=(at == AT - 1),
                        )
                    nc.scalar.activation(
                        out=y_sb[:, 2 * pr : 2 * pr + 2, :], in_=y_ps,
                        func=mybir.ActivationFunctionType.Copy)

                ag_in = dp.tile([BL * Ci], f16, tag="agin")
                nc.scalar.dma_start(
                    out=ag_in.rearrange("(o n) -> o n", o=1),
                    in_=y_sb.rearrange("p b s -> p (b s)"))
                ag_out = dp.tile([B * Ci], f16, tag="agout",
                                 addr_space="Shared")
                nc.gpsimd.collective_compute(
                    "AllGather", mybir.AluOpType.bypass,
                    ins=[ag_in[:]], outs=[ag_out[:]],
                    replica_groups=[list(range(NCORES))],
                )
                y32 = sp.tile([B + 1, Ci], f16, tag="y32", bufs=3)
                nc.gpsimd.memset(y32[B : B + 1, :], 1.0)
                nc.scalar.dma_start(
                    out=y32[:B, :], in_=ag_out.rearrange("(j s) -> j s", s=Ci))
                if debug_taps:
                    nc.scalar.dma_start(out=ydbg[:, off : off + Ci],
                                        in_=y32[:B, :])
                return dict(hi_sb=hi_sb, hit_sb=hit_sb, y32=y32,
                            i=i, Ci=Ci)

            def load_sel(c):
                i, Ci = c["i"], c["Ci"]
                off = offs[i]
                sel_c = sp.tile([B + 1, BL, Ci], f16, tag="selc", bufs=3)
                nc.scalar.dma_start(out=sel_c, in_=sel[:, :, off : off + Ci])
                c["sel_c"] = sel_c

            def phase2(c):
                i, Ci = c["i"], c["Ci"]
                TL = Ci // 128
                sel_c, y32 = c["sel_c"], c["y32"]
                # masked_scatter selection: one-hot rows (plus a -60000 mask
                # row) dotted with [y; 1]
                selY = sp.tile([B + 1, BL, Ci], f16, tag="selY", bufs=2)
                nc.vector.tensor_mul(
                    selY, sel_c,
                    y32.rearrange("j (o s) -> j o s", o=1)
                       .broadcast_to([B + 1, BL, Ci]))
                bts = []
                for pr in range(BL // 2):
                    bt_ps = py.tile([1, 2, Ci], f32, tag="y")
                    nc.tensor.matmul(
                        bt_ps, ones_sb,
                        selY[:, 2 * pr : 2 * pr + 2, :],
                        start=True, stop=True)
                    bts.append(bt_ps)
                if debug_taps:
                    off = offs[i]
                    btcp = sp.tile([1, BL, Ci], f32, tag="btcp", bufs=2)
                    for pr in range(BL // 2):
                        nc.vector.tensor_copy(
                            btcp[:, 2 * pr : 2 * pr + 2, :], bts[pr])
                    nc.scalar.dma_start(
                        out=btdbg[:, :, off : off + Ci], in_=btcp)
                if i == 0:
                    # fix the per-batch shift from the first chunk's max
                    cmax0 = sp.tile([1, BL], f32, tag="cmax0", bufs=1)
                    for pr in range(BL // 2):
                        nc.vector.tensor_reduce(
                            out=cmax0[:, 2 * pr : 2 * pr + 2]
                                .rearrange("p (b o) -> p b o", o=1),
                            in_=bts[pr],
                            axis=mybir.AxisListType.X, op=mybir.AluOpType.max)
                    nc.vector.tensor_scalar(
                        nCb, cmax0, -1.0, -float(MARG),
                        op0=mybir.AluOpType.mult, op1=mybir.AluOpType.add)
                e_g = sp.tile([1, BL, Ci], f16, tag="eg", bufs=3)
                for b in range(BL):
                    nc.scalar.activation(
                        out=e_g[:, b, :], in_=bts[b // 2][:, b % 2, :],
                        func=mybir.ActivationFunctionType.Exp,
                        bias=nCb[:, b : b + 1], scale=1.0,
                        accum_out=lall[:, b, i : i + 1])
                e_bc = ebp.tile([128, BL, Ci], f16, tag="ebc")
                nc.gpsimd.partition_broadcast(
                    e_bc.rearrange("p b s -> p (b s)"),
                    e_g.rearrange("p b s -> p (b s)"))
                c["ebc"] = e_bc
                c["eg"] = e_g

            def phase3(c):
                i, Ci = c["i"], c["Ci"]
                TL = Ci // 128
                st0 = offs[i] // 128
                sacc_i = saccs[i]
                ttr_scr = sp.tile([128, Ci], f16, tag="ttrscr", bufs=1)
                hi_sb, hit_sb = c["hi_sb"], c["hit_sb"]
                e_bc_all = c["ebc"]
                # transpose e onto partitions (outer product with [1] ones);
                # done here so the PE never waits on phase 2's exp
                e_g = c["eg"]
                eT_ps = py.tile([128, TL, BL], f32, tag="y")
                for tl in range(TL):
                    for b in range(BL):
                        nc.tensor.matmul(
                            eT_ps[:, tl, b : b + 1],
                            e_g[:, b, tl * 128 : (tl + 1) * 128],
                            one1,
                            start=True, stop=True)
                nc.vector.tensor_copy(eT_sb[:, st0 : st0 + TL, :], eT_ps)
                for b in range(BL):
                    e_bc = e_bc_all[:, b, :]
                    for kt in range(J, KT):
                        nc.vector._custom_dve(
                            TENSOR_TENSOR_REDUCE,
                            out=ttr_scr,
                            in0=hi_sb[:, kt, b, :],
                            in1=e_bc,
                            s0=0.0, s1=1.0,
                            accum_out=sacc_i[:, kt - J, b : b + 1],
                        )
                    for tl in range(TL):
                        st = st0 + tl
                        nc.tensor.matmul(
                            mv[b],
                            eT_sb[:, st, b : b + 1],
                            hit_sb[:, tl, b, :],
                            start=(st == 0), stop=(st == ST - 1),
                            skip_group_check=True,
                        )

            # phase2 lags phase1 by 2 chunks (AllGather slack off the PE
            # stream), phase3 by 3.
            cs = {}
            for i in range(NCH):
                cs[i] = phase1(i)
                if i >= 1:
                    load_sel(cs[i - 1])
                if i >= 2:
                    phase2(cs[i - 2])
                    phase3(cs[i - 2])
            load_sel(cs[NCH - 1])
            phase2(cs[NCH - 2])
            phase3(cs[NCH - 2])
            phase2(cs[NCH - 1])
            phase3(cs[NCH - 1])

            # ---- finalize: s / l, both halves
            lsum = sp.tile([1, BL], f32, tag="lsum")
            nc.vector.tensor_reduce(
                out=lsum.rearrange("p (b o) -> p b o", o=1), in_=lall,
                axis=mybir.AxisListType.X, op=mybir.AluOpType.add)
            il = sp.tile([1, BL], f32, tag="il")
            nc.vector.reciprocal(il, lsum)
            # PE half: drain the persistent accumulators, scaled by 1/l
            for b in range(BL):
                mv_sb = sp.tile([1, HJ], f32, tag="mvsb", bufs=1)
                nc.scalar.activation(
                    out=mv_sb, in_=mv[b],
                    func=mybir.ActivationFunctionType.Copy,
                    bias=0.0, scale=il[:, b : b + 1])
                nc.sync.dma_start(out=out[b : b + 1, 0:HJ], in_=mv_sb)
            # DVE half: sum chunk partials, scale, transpose, store
            ssum = sp.tile([128, KTD, BL], f32, tag="ssum")
            nc.vector.tensor_add(ssum, saccs[0], saccs[1])
            for i in range(2, NCH):
                nc.vector.tensor_add(ssum, ssum, saccs[i])
            wbc = ebp.tile([128, BL], f32, tag="wbc")
            nc.gpsimd.partition_broadcast(wbc, il)
            sfin = sp.tile([128, KTD, BL], f32, tag="sfin")
            for b in range(BL):
                nc.vector.tensor_scalar_mul(
                    sfin[:, :, b], ssum[:, :, b], wbc[:, b : b + 1])
            t_ps = py.tile([KTD * BL, 128], f32, tag="y")
            nc.tensor.transpose(
                t_ps, sfin.rearrange("p t b -> p (t b)"), ident)
            t_sb = sp.tile([KTD * BL, 128], f32, tag="tsb")
            nc.vector.tensor_copy(t_sb, t_ps)
            for t in range(KTD):
                nc.sync.dma_start(
                    out=out[:, (J + t) * 128 : (J + t + 1) * 128],
                    in_=t_sb[t * BL : (t + 1) * BL, :])

    nc.compile()
    _split_pe_waits(nc)
    return nc


def _split_pe_waits(nc):
    """TRN2 PE instructions (S3_LW encoding) take a single sync-wait slot.
    Bacc's legalization misses some Matmults; hoist excess waits onto
    dedicated PE NoOps inserted directly before the offender."""
    for f in nc.m.functions:
        for bb in f.blocks:
            insts = bb.instructions
            i = 0
            while i < len(insts):
                ins = insts[i]
                if type(ins).__name__ in ("InstMatmult", "InstNoOp") and \
                        ins.engine == mybir.EngineType.PE:
                    si = ins.sync_info
                    if si is not None and len(si.on_wait) > 1:
                        extra, keep = si.on_wait[:-1], si.on_wait[-1:]
                        for w in extra:
                            nop = mybir.InstNoOp(
                                name=nc.get_next_instruction_name(),
                                ins=[], outs=[])
                            nop.engine = ins.engine
                            nop.sync_info = mybir.SyncInfo(
                                on_wait=[w], on_update=[])
                            nc.register_instruction(nop)
                            insts.insert(i, nop)
                            i += 1
                        si.on_wait = keep
                i += 1


def prep_inputs(h_i, h_t, mask, W, b, u, S=S, H=H, A=A, C=256, J=4,
                clist=None):
    """Shard + lay out the full inputs for the 8 cores."""
    h_i = np.asarray(h_i, np.float32)
    h_t = np.asarray(h_t, np.float32)
    mask = np.asarray(mask, bool)
    W = np.asarray(W, np.float32)
    b = np.asarray(b, np.float32)
    u = np.asarray(u, np.float32)

    KT = H // 128
    AT = A // 128
    HJ = J * 128
    ST = S // 128
    if clist is None:
        clist = [C] * (S // C)
    offs = np.concatenate([[0], np.cumsum(clist)]).astype(int)
    w1t = np.ascontiguousarray(W[:, :H].T).astype(np.float16)   # [H, A]
    cb = h_t @ W[:, H:].T + b                                   # [B, A]
    cb2s = np.ascontiguousarray(
        cb.reshape(B, AT, 128).transpose(2, 1, 0))              # [128, AT, B]
    u2 = np.ascontiguousarray(
        u[:, 0].reshape(AT, 128).T).astype(np.float16)          # [128, AT]

    pos = np.clip(np.cumsum(mask.astype(np.int64), axis=0) - 1, 0, None)
    onehot = (np.arange(B)[None, :, None] == pos[:, None, :]) & mask[:, None, :]
    selall = onehot.astype(np.float16)                          # [B, B, S]
    negall = np.where(mask, np.float16(0), NEG).astype(np.float16)  # [B, S]
    sel33 = np.concatenate([selall, negall[:, None, :]], axis=1)  # [B, B+1, S]

    h16 = h_i.astype(np.float16)
    in_maps = []
    for c in range(NCORES):
        bs = slice(c * BL, (c + 1) * BL)
        # hi5[p, block_i ++ (t, b, s)] = h_i[b, off_i+s, t*128+p]
        hcf = h16[bs].reshape(BL, S, KT, 128)
        blocks = []
        for ci, off in zip(clist, offs[:-1]):
            hc = hcf[:, off : off + ci]                     # [BL, ci, KT, 128]
            blocks.append(hc.transpose(3, 2, 0, 1).reshape(128, KT * BL * ci))
        hi5 = np.ascontiguousarray(np.concatenate(blocks, axis=1))
        # hit5[p, (st, b, h')] = h_i[b, st*128+p, h']  for h' < HJ
        hh = h16[bs][:, :, :HJ].reshape(BL, ST, 128, HJ)
        hit5 = np.ascontiguousarray(
            hh.transpose(2, 1, 0, 3).reshape(128, ST * BL * HJ))
        in_maps.append({
            "hi5": hi5,
            "hit5": hit5,
            "w1t": w1t,
            "cb2": np.ascontiguousarray(cb2s[:, :, bs]),
            "u2": u2,
            "sel": np.ascontiguousarray(sel33[bs].transpose(1, 0, 2)),
        })
    return in_maps


_NC_CACHE = {}


CLIST = [256] * 8


def _get_nc():
    if "nc" not in _NC_CACHE:
        _NC_CACHE["nc"] = build_kernel(clist=CLIST)
    return _NC_CACHE["nc"]


def kernel(h_i, h_t, mask, W, b, u):
    nc = _get_nc()
    in_maps = prep_inputs(h_i, h_t, mask, W, b, u, clist=CLIST)
    res = run_bass_kernel_spmd(nc, in_maps, list(range(NCORES)))
    return np.concatenate([res.results[c]["out"] for c in range(NCORES)],
                          axis=0)


# revision 30
# speedup vs baseline: 1.3533x; 1.3533x over previous
"""Trainium2 Bass kernel for nn_Attention_b (tanh-attention with masked_scatter).

Data-parallel over batch: each of 8 NeuronCores owns 4 batches. Per core:
  phase 1  z = W1 @ h_i + (W2 @ h_t + b)   (fp16 GEMM, [A, rows])
           m = tanh(z); y = u . m          (raw scores, [rows])
  comm     AllGather of the per-chunk score slice across the 8 cores
  phase 2  masked_scatter selection (0/1 matrix against gathered scores),
           e = exp(beta - C_b) with a per-batch shift C_b fixed from the
           first chunk's max (no flash recurrence needed)
  phase 3  weighted sum s += e * h_i, split across engines:
             h' in [0, 512):    PE matvec on DMA-transposed h_i tiles,
                                accumulated in persistent PSUM banks
             h' in [512, 1024): fused DVE multiply-reduce on the resident
                                GEMM-layout h_i chunk
"""
import sys

for _p in ("/opt/trn_rl_repo",):
    if _p not in sys.path:
        sys.path.insert(0, _p)

import numpy as np

import concourse.bacc as bacc
import concourse.tile as tile
from concourse import mybir
from concourse.bass_utils import run_bass_kernel_spmd
from concourse.dve_ops import TENSOR_TENSOR_REDUCE
from concourse.masks import make_identity

NCORES = 8
B, S, H, A = 32, 2048, 1024, 256
BL = B // NCORES          # local batches per core
NEG = np.float32(-60000.0)   # fp16-representable "minus infinity"
MARG = np.float32(4.0)       # shift margin over the chunk-0 max

f32 = mybir.dt.float32
f16 = mybir.dt.float16


def build_kernel(S=S, H=H, A=A, C=256, J=4, hi_bufs=7, hit_bufs=6,
                 clist=None, debug_taps=False):
    KT = H // 128             # contraction tiles
    AT = A // 128             # score tiles
    HJ = J * 128              # h-dims handled by the PE matvec half
    KTD = KT - J              # h-tiles handled by the DVE half
    ST = S // 128             # s-tiles for the transposed layout
    if clist is None:
        clist = [C] * (S // C)
    offs = np.concatenate([[0], np.cumsum(clist)]).tolist()
    NCH = len(clist)
    assert offs[-1] == S and H % 128 == 0 and A % 128 == 0
    assert all(c % 128 == 0 for c in clist)

    nc = bacc.Bacc("TRN2", target_bir_lowering=False, debug=False,
                   num_devices=NCORES)

    hi5 = nc.declare_dram_parameter("hi5", [128, KT * BL * S], f16,
                                    isOutput=False)
    hit5 = nc.declare_dram_parameter("hit5", [128, ST * BL * HJ], f16,
                                     isOutput=False)
    w1t = nc.declare_dram_parameter("w1t", [H, A], f16, isOutput=False)
    cb2 = nc.declare_dram_parameter("cb2", [128, AT, BL], f32, isOutput=False)
    u2 = nc.declare_dram_parameter("u2", [128, AT], f16, isOutput=False)
    sel = nc.declare_dram_parameter("sel", [B + 1, BL, S], f16,
                                    isOutput=False)
    out = nc.declare_dram_parameter("out", [BL, H], f32, isOutput=True)
    if debug_taps:
        ydbg = nc.declare_dram_parameter("ydbg", [B, S], f16, isOutput=True)
        btdbg = nc.declare_dram_parameter("btdbg", [1, BL, S], f32,
                                          isOutput=True)

    with tile.TileContext(nc) as tc:
        with (
            tc.tile_pool(name="consts", bufs=1) as cp,
            tc.tile_pool(name="hi", bufs=hi_bufs) as hip,
            tc.tile_pool(name="hit", bufs=hit_bufs) as hitp,
            tc.tile_pool(name="m", bufs=2) as mp,
            tc.tile_pool(name="small", bufs=3) as sp,
            tc.tile_pool(name="ebc", bufs=2) as ebp,
            tc.tile_pool(name="pz", bufs=2, space="PSUM") as pz,
            tc.tile_pool(name="py", bufs=2, space="PSUM") as py,
            tc.tile_pool(name="pm", bufs=1, space="PSUM") as pm,
            tc.tile_pool(name="dram", bufs=NCH, space="DRAM") as dp,
        ):
            # ---- dummy collective first: absorbs the comm-init barrier and
            # cross-core launch skew before any real dependency forms on it
            wu_in = dp.tile([512], f16, tag="wuin")
            wu_out = dp.tile([NCORES * 512], f16, tag="wuout",
                             addr_space="Shared")
            nc.gpsimd.collective_compute(
                "AllGather", mybir.AluOpType.bypass,
                ins=[wu_in[:]], outs=[wu_out[:]],
                replica_groups=[list(range(NCORES))],
            )

            # ---- preload replicated constants
            w1_sb = cp.tile([128, KT, A], f16)
            nc.sync.dma_start(
                out=w1_sb, in_=w1t.rearrange("(t p) a -> p t a", p=128))
            u_sb = cp.tile([128, AT], f16)
            nc.sync.dma_start(out=u_sb, in_=u2[:, :])
            cb_sb = cp.tile([128, AT, BL], f32)
            nc.sync.dma_start(out=cb_sb, in_=cb2[:, :, :])
            ident = cp.tile([128, 128], f32)
            make_identity(nc, ident)
            ones_sb = cp.tile([B + 1, 1], f16)
            nc.vector.memset(ones_sb, 1.0)
            one1 = cp.tile([1, 1], f16)
            nc.vector.memset(one1, 1.0)

            # per-chunk softmax l-partials; shared per-batch shift
            lall = cp.tile([1, BL, NCH], f32)
            nc.vector.memset(lall, 0.0)
            nCb = cp.tile([1, BL], f32)
            nc.vector.memset(nCb, 0.0)
            eT_sb = cp.tile([128, ST, BL], f16)
            saccs = [cp.tile([128, KTD, BL], f32, name=f"sacc{i}")
                     for i in range(NCH)]
            mv = [pm.tile([1, HJ], f32, name=f"mv{b}", tag=f"mv{b}")
                  for b in range(BL)]

            carries = []

            def phase1(i):
                Ci, off = clist[i], offs[i]
                TL = Ci // 128
                hi_sb = hip.tile([128, KT, BL, Ci], f16, tag="hi")
                nc.sync.dma_start(
                    out=hi_sb.rearrange("p t b s -> p (t b s)"),
                    in_=hi5[:, KT * BL * off : KT * BL * (off + Ci)])
                hit_sb = hitp.tile([128, TL, BL, HJ], f16, tag="hit")
                st0 = off // 128
                nc.sync.dma_start(
                    out=hit_sb.rearrange("p t b h -> p (t b h)"),
                    in_=hit5[:, st0 * BL * HJ : (st0 + TL) * BL * HJ])
                m_r = mp.tile([128, AT, BL, Ci], f16, tag="m")
                for at in range(AT):
                    for pr in range(BL // 2):
                        z_ps = pz.tile([128, 2, Ci], f32, tag="z")
                        for kt in range(KT):
                            nc.tensor.matmul(
                                z_ps,
                                w1_sb[:, kt, at * 128 : (at + 1) * 128],
                                hi_sb[:, kt, 2 * pr : 2 * pr + 2, :],
                                start=(kt == 0), stop=(kt == KT - 1),
                            )
                        for bh in range(2):
                            b = 2 * pr + bh
                            nc.scalar.activation(
                                out=m_r[:, at, b, :], in_=z_ps[:, bh, :],
                                func=mybir.ActivationFunctionType.Tanh,
                                bias=cb_sb[:, at, b : b + 1], scale=1.0,
                            )
                y_sb = sp.tile([1, BL, Ci], f16, tag="ysb", bufs=2)
                for pr in range(BL // 2):
                    y_ps = py.tile([1, 2, Ci], f32, tag="y")
                    for at in range(AT):
                        nc.tensor.matmul(
                            y_ps,
                            u_sb[:, at : at + 1],
                            m_r[:, at, 2 * pr : 2 * pr + 2, :],
                            start=(at == 0), stop=(at == AT - 1),
                        )
                    nc.scalar.activation(
                        out=y_sb[:, 2 * pr : 2 * pr + 2, :], in_=y_ps,
                        func=mybir.ActivationFunctionType.Copy)

                ag_in = dp.tile([BL * Ci], f16, tag="agin")
                nc.scalar.dma_start(
                    out=ag_in.rearrange("(o n) -> o n", o=1),
                    in_=y_sb.rearrange("p b s -> p (b s)"))
                ag_out = dp.tile([B * Ci], f16, tag="agout",
                                 addr_space="Shared")
                nc.gpsimd.collective_compute(
                    "AllGather", mybir.AluOpType.bypass,
                    ins=[ag_in[:]], outs=[ag_out[:]],
                    replica_groups=[list(range(NCORES))],
                )
                y32 = sp.tile([B + 1, Ci], f16, tag="y32", bufs=3)
                nc.gpsimd.memset(y32[B : B + 1, :], 1.0)
                nc.scalar.dma_start(
                    out=y32[:B, :], in_=ag_out.rearrange("(j s) -> j s", s=Ci))
                if debug_taps:
                    nc.scalar.dma_start(out=ydbg[:, off : off + Ci],
                                        in_=y32[:B, :])
                return dict(hi_sb=hi_sb, hit_sb=hit_sb, y32=y32,
                            i=i, Ci=Ci)

            def load_sel(c):
                i, Ci = c["i"], c["Ci"]
                off = offs[i]
                sel_c = sp.tile([B + 1, BL, Ci], f16, tag="selc", bufs=3)
                nc.scalar.dma_start(out=sel_c, in_=sel[:, :, off : off + Ci])
                c["sel_c"] = sel_c

            def phase2(c):
                i, Ci = c["i"], c["Ci"]
                TL = Ci // 128
                sel_c, y32 = c["sel_c"], c["y32"]
                # masked_scatter selection: one-hot rows (plus a -60000 mask
                # row) dotted with [y; 1]
                selY = sp.tile([B + 1, BL, Ci], f16, tag="selY", bufs=2)
                nc.vector.tensor_mul(
                    selY, sel_c,
                    y32.rearrange("j (o s) -> j o s", o=1)
                       .broadcast_to([B + 1, BL, Ci]))
                bts = []
                for pr in range(BL // 2):
                    bt_ps = py.tile([1, 2, Ci], f32, tag="y")
                    nc.tensor.matmul(
                        bt_ps, ones_sb,
                        selY[:, 2 * pr : 2 * pr + 2, :],
                        start=True, stop=True)
                    bts.append(bt_ps)
                if debug_taps:
                    off = offs[i]
                    btcp = sp.tile([1, BL, Ci], f32, tag="btcp", bufs=2)
                    for pr in range(BL // 2):
                        nc.vector.tensor_copy(
                            btcp[:, 2 * pr : 2 * pr + 2, :], bts[pr])
                    nc.scalar.dma_start(
                        out=btdbg[:, :, off : off + Ci], in_=btcp)
                if i == 0:
                    # fix the per-batch shift from the first chunk's max
                    cmax0 = sp.tile([1, BL], f32, tag="cmax0", bufs=1)
                    for pr in range(BL // 2):
                        nc.vector.tensor_reduce(
                            out=cmax0[:, 2 * pr : 2 * pr + 2]
                                .rearrange("p (b o) -> p b o", o=1),
                            in_=bts[pr],
                            axis=mybir.AxisListType.X, op=mybir.AluOpType.max)
                    nc.vector.tensor_scalar(
                        nCb, cmax0, -1.0, -float(MARG),
                        op0=mybir.AluOpType.mult, op1=mybir.AluOpType.add)
                e_g = sp.tile([1, BL, Ci], f16, tag="eg", bufs=3)
                for b in range(BL):
                    nc.scalar.activation(
                        out=e_g[:, b, :], in_=bts[b // 2][:, b % 2, :],
                        func=mybir.ActivationFunctionType.Exp,
                        bias=nCb[:, b : b + 1], scale=1.0,
                        accum_out=lall[:, b, i : i + 1])
                e_bc = ebp.tile([128, BL, Ci], f16, tag="ebc")
                nc.gpsimd.partition_broadcast(
                    e_bc.rearrange("p b s -> p (b s)"),
                    e_g.rearrange("p b s -> p (b s)"))
                c["ebc"] = e_bc
                c["eg"] = e_g

            def phase3(c):
                i, Ci = c["i"], c["Ci"]
                TL = Ci // 128
                st0 = offs[i] // 128
                sacc_i = saccs[i]
                ttr_scr = sp.tile([128, Ci], f16, tag="ttrscr", bufs=1)
                hi_sb, hit_sb = c["hi_sb"], c["hit_sb"]
                e_bc_all = c["ebc"]
                # transpose e onto partitions (outer product with [1] ones);
                # done here so the PE never waits on phase 2's exp
                e_g = c["eg"]
                eT_ps = py.tile([128, TL, BL], f32, tag="y")
                for tl in range(TL):
                    for b in range(BL):
                        nc.tensor.matmul(
                            eT_ps[:, tl, b : b + 1],
                            e_g[:, b, tl * 128 : (tl + 1) * 128],
                            one1,
                            start=True, stop=True)
                nc.vector.tensor_copy(eT_sb[:, st0 : st0 + TL, :], eT_ps)
                for b in range(BL):
                    e_bc = e_bc_all[:, b, :]
                    for kt in range(J, KT):
                        nc.vector._custom_dve(
                            TENSOR_TENSOR_REDUCE,
                            out=ttr_scr,
                            in0=hi_sb[:, kt, b, :],
                            in1=e_bc,
                            s0=0.0, s1=1.0,
                            accum_out=sacc_i[:, kt - J, b : b + 1],
                        )
                    for tl in range(TL):
                        st = st0 + tl
                        nc.tensor.matmul(
                            mv[b],
                            eT_sb[:, st, b : b + 1],
                            hit_sb[:, tl, b, :],
                            start=(st == 0), stop=(st == ST - 1),
                            skip_group_check=True,
                        )

            # phase2 lags phase1 by 2 chunks (AllGather slack off the PE
            # stream), phase3 by 3.
            cs = {}
            for i in range(NCH):
                cs[i] = phase1(i)
                if i >= 1:
                    load_sel(cs[i - 1])
                if i >= 2:
                    phase2(cs[i - 2])
                    phase3(cs[i - 2])
            load_sel(cs[NCH - 1])
            phase2(cs[NCH - 2])
            phase3(cs[NCH - 2])
            phase2(cs[NCH - 1])
            phase3(cs[NCH - 1])

            # ---- finalize: s / l, both halves
            lsum = sp.tile([1, BL], f32, tag="lsum")
            nc.vector.tensor_reduce(
                out=lsum.rearrange("p (b o) -> p b o", o=1), in_=lall,
                axis=mybir.AxisListType.X, op=mybir.AluOpType.add)
            il = sp.tile([1, BL], f32, tag="il")
            nc.vector.reciprocal(il, lsum)
            # PE half: drain the persistent accumulators, scaled by 1/l
            for b in range(BL):
                mv_sb = sp.tile([1, HJ], f32, tag="mvsb", bufs=1)
                nc.scalar.activation(
                    out=mv_sb, in_=mv[b],
                    func=mybir.ActivationFunctionType.Copy,
                    bias=0.0, scale=il[:, b : b + 1])
                nc.sync.dma_start(out=out[b : b + 1, 0:HJ], in_=mv_sb)
            # DVE half: sum chunk partials, scale, transpose, store
            ssum = sp.tile([128, KTD, BL], f32, tag="ssum")
            nc.vector.tensor_add(ssum, saccs[0], saccs[1])
            for i in range(2, NCH):
                nc.vector.tensor_add(ssum, ssum, saccs[i])
            wbc = ebp.tile([128, BL], f32, tag="wbc")
            nc.gpsimd.partition_broadcast(wbc, il)
            sfin = sp.tile([128, KTD, BL], f32, tag="sfin")
            for b in range(BL):
                nc.vector.tensor_scalar_mul(
                    sfin[:, :, b], ssum[:, :, b], wbc[:, b : b + 1])
            t_ps = py.tile([KTD * BL, 128], f32, tag="y")
            nc.tensor.transpose(
                t_ps, sfin.rearrange("p t b -> p (t b)"), ident)
            t_sb = sp.tile([KTD * BL, 128], f32, tag="tsb")
            nc.vector.tensor_copy(t_sb, t_ps)
            for t in range(KTD):
                nc.sync.dma_start(
                    out=out[:, (J + t) * 128 : (J + t + 1) * 128],
                    in_=t_sb[t * BL : (t + 1) * BL, :])

    nc.compile()
    _split_pe_waits(nc)
    return nc


def _split_pe_waits(nc):
    """TRN2 PE instructions (S3_LW encoding) take a single sync-wait slot.
    Bacc's legalization misses some Matmults; hoist excess waits onto
    dedicated PE NoOps inserted directly before the offender."""
    for f in nc.m.functions:
        for bb in f.blocks:
            insts = bb.instructions
            i = 0
            while i < len(insts):
                ins = insts[i]
                if type(ins).__name__ in ("InstMatmult", "InstNoOp") and \
                        ins.engine == mybir.EngineType.PE:
                    si = ins.sync_info
                    if si is not None and len(si.on_wait) > 1:
                        extra, keep = si.on_wait[:-1], si.on_wait[-1:]
                        for w in extra:
                            nop = mybir.InstNoOp(
                                name=nc.get_next_instruction_name(),
                                ins=[], outs=[])
                            nop.engine = ins.engine
                            nop.sync_info = mybir.SyncInfo(
                                on_wait=[w], on_update=[])
                            nc.register_instruction(nop)
                            insts.insert(i, nop)
                            i += 1
                        si.on_wait = keep
                i += 1


def prep_inputs(h_i, h_t, mask, W, b, u, S=S, H=H, A=A, C=256, J=4,
                clist=None):
    """Shard + lay out the full inputs for the 8 cores."""
    h_i = np.asarray(h_i, np.float32)
    h_t = np.asarray(h_t, np.float32)
    mask = np.asarray(mask, bool)
    W = np.asarray(W, np.float32)
    b = np.asarray(b, np.float32)
    u = np.asarray(u, np.float32)

    KT = H // 128
    AT = A // 128
    HJ = J * 128
    ST = S // 128
    if clist is None:
        clist = [C] * (S // C)
    offs = np.concatenate([[0], np.cumsum(clist)]).astype(int)
    w1t = np.ascontiguousarray(W[:, :H].T).astype(np.float16)   # [H, A]
    cb = h_t @ W[:, H:].T + b                                   # [B, A]
    cb2s = np.ascontiguousarray(
        cb.reshape(B, AT, 128).transpose(2, 1, 0))              # [128, AT, B]
    u2 = np.ascontiguousarray(
        u[:, 0].reshape(AT, 128).T).astype(np.float16)          # [128, AT]

    pos = np.clip(np.cumsum(mask.astype(np.int64), axis=0) - 1, 0, None)
    onehot = (np.arange(B)[None, :, None] == pos[:, None, :]) & mask[:, None, :]
    selall = onehot.astype(np.float16)                          # [B, B, S]
    negall = np.where(mask, np.float16(0), NEG).astype(np.float16)  # [B, S]
    sel33 = np.concatenate([selall, negall[:, None, :]], axis=1)  # [B, B+1, S]

    h16 = h_i.astype(np.float16)
    in_maps = []
    for c in range(NCORES):
        bs = slice(c * BL, (c + 1) * BL)
        # hi5[p, block_i ++ (t, b, s)] = h_i[b, off_i+s, t*128+p]
        hcf = h16[bs].reshape(BL, S, KT, 128)
        blocks = []
        for ci, off in zip(clist, offs[:-1]):
            hc = hcf[:, off : off + ci]                     # [BL, ci, KT, 128]
            blocks.append(hc.transpose(3, 2, 0, 1).reshape(128, KT * BL * ci))
        hi5 = np.ascontiguousarray(np.concatenate(blocks, axis=1))
        # hit5[p, (st, b, h')] = h_i[b, st*128+p, h']  for h' < HJ
        hh = h16[bs][:, :, :HJ].reshape(BL, ST, 128, HJ)
        hit5 = np.ascontiguousarray(
            hh.transpose(2, 1, 0, 3).reshape(128, ST * BL * HJ))
        in_maps.append({
            "hi5": hi5,
            "hit5": hit5,
            "w1t": w1t,
            "cb2": np.ascontiguousarray(cb2s[:, :, bs]),
            "u2": u2,
            "sel": np.ascontiguousarray(sel33[bs].transpose(1, 0, 2)),
        })
    return in_maps


_NC_CACHE = {}


CLIST = [256] * 8


def _get_nc():
    if "nc" not in _NC_CACHE:
        _NC_CACHE["nc"] = build_kernel(clist=CLIST)
    return _NC_CACHE["nc"]


def kernel(h_i, h_t, mask, W, b, u):
    nc = _get_nc()
    in_maps = prep_inputs(h_i, h_t, mask, W, b, u, clist=CLIST)
    res = run_bass_kernel_spmd(nc, in_maps, list(range(NCORES)))
    return np.concatenate([res.results[c]["out"] for c in range(NCORES)],
                          axis=0)
